# revision 52
# baseline (speedup 1.0000x reference)
"""AttnBlock (GroupNorm -> single-head 4096x4096 attention -> proj -> residual)
on x:[2,512,64,64] f32, distributed over 8 trn2 NeuronCores.

Sharding: data-parallel over batch (2) x sequence-parallel over query rows
(4 chunks of 1024). Each core receives its batch's full [512, 4096] image with
spatial columns permuted so that its own 1024 query positions are columns
0:1024 (attention and groupnorm are permutation-invariant over spatial
positions, which keeps the SPMD program identical across cores).

Precision ladder (fp32 accumulation in PSUM throughout): x streams in as
bf16 (half the head DMA) and the convs/proj run bf16; the attention
operands K/Q/V^T/exp(S) are fp8 e4m3, so the two dominant matmuls (S=K^T Q
and O=V^T P, ~2/3 of all MACs) run as DoubleRow matmuls -- 256-deep
contraction per instruction at the fp8 2x rate -- and V^T stays
SBUF-resident (no DRAM spill/reload). exp is computed as exp(s*scale - 5):
softmax is invariant to the row-constant shift and it keeps exp outputs
inside e4m3 range. A numpy simulation of this quantization through the
reference gives 3.8e-3 absmax relative error (gate is 2e-2). The residual
path reads a separate fp32 copy of the core's own 1024 columns.

The x DMA is striped over three queues (sync/gpsimd/scalar) with all issue
instructions emitted before any compute op, weights queue behind the x
slices, and the late-needed residual/proj-weight transfers are gated behind
the last x slice via a tiny ACT dependency op. GroupNorm stats are split
DVE (bn_stats, 6 slices) / ACT (Square+Copy accum_out, first 2 slices to
arrive), combined as raw sums with the group-average and 1/N normalization
folded into the host-built gmat.

GroupNorm is folded into the q/k/v weights on device (h = (x-mu)*rstd, so
W^T rows are scaled by rstd). Bias handling exploits softmax structure:
  - K bias (and its -W^T mu correction) adds a per-row constant to the
    logits -> cancels in softmax -> skipped entirely.
  - V bias is a per-channel constant on the attention output -> folded into
    the proj bias ON HOST (bp += wp @ bv); only the runtime -W_v^T mu part
    is applied on device, pushed through wp into the final bias with 16
    tiny transposed matmuls.
  - Q bias is applied at Q eviction; its correction (bq - W_q^T mu) is
    computed with 16 tiny transposed matmuls directly in per-partition
    layout (no DRAM-roundtrip transpose).
Softmax normalization: exp tiles accumulate on DVE into bf16 row-partials;
one ones[128,128] matmul broadcasts the cross-partition row sums to all
partitions, and a [128,512] DVE reciprocal feeds the final scale -- all off
the PE critical path.
"""

import numpy as np

import concourse.bass as bass
import concourse.mybir as mybir
import concourse.tile as tile
from concourse import bacc
from concourse.bass_utils import run_bass_kernel_spmd

F32 = mybir.dt.float32
BF16 = mybir.dt.bfloat16
FP8 = mybir.dt.float8e4

B = 2
C = 512
H = 64
W = 64
N = H * W            # 4096 spatial positions
G = 32               # groups
EPS = 1e-6
CH = 4               # channel chunks of 128
NS = 8               # j slices of 512
JT = 32              # j tiles of 128
I = 1024             # query positions per core
IC = 2               # i chunks of 512 per core
SCALE = float(C) ** -0.5

_cached = {}


def _build(repeat=1):
    nc = bacc.Bacc("TRN2", target_bir_lowering=False, debug=False, num_devices=8)

    # all inputs are host-relayouted to partition-major so every DMA is a
    # straight [128, k] copy with fully contiguous per-partition lines
    x_d = nc.dram_tensor("x", [128, NS, CH, 512], BF16, kind="ExternalInput").ap()
    x8_d = nc.dram_tensor("x8", [128, NS, CH, 512], FP8, kind="ExternalInput").ap()
    x32_d = nc.dram_tensor("x32", [128, CH, I], F32, kind="ExternalInput").ap()
    wq_d = nc.dram_tensor("wqt", [128, CH, C], BF16, kind="ExternalInput").ap()
    wk_d = nc.dram_tensor("wkt", [128, CH, C], BF16, kind="ExternalInput").ap()
    wv_d = nc.dram_tensor("wvt", [128, CH, C], BF16, kind="ExternalInput").ap()
    wp_d = nc.dram_tensor("wpt", [128, CH, C], BF16, kind="ExternalInput").ap()
    bq_d = nc.dram_tensor("bq", [128, CH], F32, kind="ExternalInput").ap()
    bp_d = nc.dram_tensor("bp", [128, CH], F32, kind="ExternalInput").ap()
    gm_d = nc.dram_tensor("gmat", [128, 128], F32, kind="ExternalInput").ap()
    out_ds = [
        nc.dram_tensor("out" if r == 0 else f"out{r}", [C, I], F32,
                       kind="ExternalOutput").ap()
        for r in range(repeat)
    ]

    x_r = x_d       # [128, 8, 4, 512] slice-major: 4KB lines per slice DMA
    x32_r = x32_d   # [128, 4, 1024]

    with tile.TileContext(nc) as tc:
      for rep in range(repeat):
        sfx = f"_{rep}"
        out_r = out_ds[rep].rearrange("(ch p) i -> p ch i", p=128)
        from contextlib import ExitStack
        ctx_psum = ExitStack()
        with (
            tc.tile_pool(name="consts" + sfx + sfx, bufs=1) as consts,
            tc.tile_pool(name="big" + sfx + sfx, bufs=1) as big,
            tc.tile_pool(name="stats" + sfx + sfx, bufs=1) as stats,
            tc.tile_pool(name="small" + sfx + sfx, bufs=1) as small,
            tc.tile_pool(name="fin" + sfx + sfx, bufs=2) as fin,
        ):
            # ---- persistent constants (on the vector DMA queue, off the
            # x-critical sync/gpsimd queues) ----
            bp_sb = consts.tile([128, CH], F32, tag="bp")
            nc.scalar.dma_start(out=bp_sb[:], in_=bp_d)
            bq_sb = consts.tile([128, CH], F32, tag="bq")
            nc.scalar.dma_start(out=bq_sb[:], in_=bq_d)
            gm_sb = consts.tile([128, 128], F32, tag="gm")
            nc.scalar.dma_start(out=gm_sb[:], in_=gm_d)
            eps_sb = consts.tile([128, 1], F32, tag="eps")
            nc.vector.memset(eps_sb[:], EPS)
            # global logit shift: exp(s*scale - 5) keeps exp outputs inside
            # fp8 e4m3 range (max 448; logits reach ~5 sigma). Softmax is
            # invariant to a row-constant shift and the uniform e^-5 factor
            # cancels exactly in the rowsum normalization.
            expb = consts.tile([128, 1], F32, tag="expb")
            nc.vector.memset(expb[:], -5.0)
            ones128 = consts.tile([128, 128], BF16, tag="ones128")
            nc.vector.memset(ones128[:], 1.0)
            tscr = consts.tile([128, 1], F32, tag="tscr")

            X_tiles = [
                big.tile([128, CH, 512], BF16, tag=f"X{s}", name=f"X{s}" + sfx)
                for s in range(NS)
            ]  # bf16 x, resident, one tile per j-slice for fine-grained deps
            X32_sb = big.tile([128, CH, I], F32, tag="X32")  # fp32 residual slice
            # attention storage is fp8 e4m3: S and O run as DoubleRow matmuls
            # (256-deep contraction per instruction, half the PE cycles);
            # numpy sim of this quantization: 3.8e-3 absmax rel (gate 2e-2)
            K_sb = big.tile([128, CH, N], FP8, tag="K")      # K[c, j]
            Q_sb = big.tile([128, CH, I], FP8, tag="Q")      # Q[c, i]
            VT_sb = big.tile([128, JT, C], FP8, tag="VT")    # V^T[j, c] resident

            # groupnorm stats split across three engines, ownership matched
            # to DMA arrival order (slices 0/1/2 land first, one per queue):
            # ACT 2-pass accum on (0,3), GpSimd STT accum on (2,), DVE
            # bn_stats on the rest
            OWN_ACT = {0: 0, 1: 1}        # slice -> accumulator slot
            OWN_GP = {}
            NSV = NS - len(OWN_ACT) - len(OWN_GP)
            NSA = len(OWN_ACT) + len(OWN_GP)
            st_acc = stats.tile([128, CH, NSV, 6], F32, tag="stacc")
            sma = stats.tile([128, CH, NSA], F32, tag="sma")
            sqa = stats.tile([128, CH, NSA], F32, tag="sqa")
            sm2 = stats.tile([128, CH, 1], F32, tag="sm2")
            sq2 = stats.tile([128, CH, 1], F32, tag="sq2")
            scr = stats.tile([128, 512], BF16, tag="scr")
            scrg = stats.tile([128, 512], BF16, tag="scrg")
            mv = stats.tile([128, CH, 2], F32, tag="mv")
            q1 = stats.tile([128, CH], F32, tag="q1")
            gs = stats.tile([128, CH, 2], F32, tag="gs")
            mean_sb = stats.tile([128, CH], F32, tag="mean")
            rstd_sb = stats.tile([128, CH], F32, tag="rstd")

            psc = ctx_psum.enter_context(
                tc.tile_pool(name="psc" + sfx + sfx, bufs=4, space="PSUM")
            )
            with tc.tile_pool(name="wtmp" + sfx + sfx, bufs=1) as wtmp:
                # ---- phase 1: groupnorm stats over streaming bf16 x ----
                # x is striped over three DMA queues so it gets the full HBM
                # bandwidth; everything else queues behind the x slices
                nsv_i = 0
                qs = (nc.sync, nc.gpsimd, nc.scalar)
                # issue every x-slice DMA first: each engine's issue runs
                # before any of its (slow) stats ops, so the three queues
                # stream x back-to-back at full bandwidth. The fp8 conv copy
                # of x (host-quantized from the same bf16 values) queues
                # right behind the bf16 stream.
                X8 = [
                    big.tile([128, CH, 512], FP8, tag=f"X8{s}", name=f"X8{s}" + sfx)
                    for s in range(NS)
                ]
                for s in range(NS):
                    qs[s % 3].dma_start(out=X_tiles[s][:], in_=x_r[:, s, :, :])
                for s in range(NS):
                    qs[s % 3].dma_start(out=X8[s][:], in_=x8_d[:, s, :, :])
                for s in range(NS):
                    if s in OWN_ACT:
                        a = OWN_ACT[s]
                        for ch in range(CH):
                            nc.scalar.activation(
                                out=scr[:], in_=X_tiles[s][:, ch, :],
                                func=mybir.ActivationFunctionType.Square,
                                accum_out=sqa[:, ch, a:a + 1],
                            )
                            nc.scalar.activation(
                                out=scr[:], in_=X_tiles[s][:, ch, :],
                                func=mybir.ActivationFunctionType.Copy,
                                accum_out=sma[:, ch, a:a + 1],
                            )
                    elif s in OWN_GP:
                        a = OWN_GP[s]
                        for ch in range(CH):
                            nc.gpsimd.scalar_tensor_tensor(
                                out=scrg[:], in0=X_tiles[s][:, ch, :], scalar=1.0,
                                in1=X_tiles[s][:, ch, :],
                                op0=mybir.AluOpType.mult, op1=mybir.AluOpType.mult,
                                accum_out=sqa[:, ch, a:a + 1],
                            )
                            nc.gpsimd.scalar_tensor_tensor(
                                out=scrg[:], in0=X_tiles[s][:, ch, :], scalar=1.0,
                                in1=ones512[:],
                                op0=mybir.AluOpType.mult, op1=mybir.AluOpType.mult,
                                accum_out=sma[:, ch, a:a + 1],
                            )
                    else:
                        for ch in range(CH):
                            nc.vector.bn_stats(
                                out=st_acc[:, ch, nsv_i, :], in_=X_tiles[s][:, ch, :],
                            )
                        nsv_i += 1
                # prewarm the Sqrt ACT table now (runs right after the last
                # ACT stats op, overlapping the DVE aggregation chain)
                nc.scalar.activation(out=tscr[:], in_=eps_sb[:],
                                     func=mybir.ActivationFunctionType.Sqrt)
                # weights queue behind the x slices on sync; the residual fp32
                # slice and proj weight queue behind the x slices on gpsimd.
                # Each queue drains in order, so x gets the full HBM bandwidth
                # first and the late-needed tensors stream under the convs.
                wk_sb4 = wtmp.tile([128, CH, C], BF16, tag="wk4")
                wv_sb4 = wtmp.tile([128, CH, C], BF16, tag="wv4")
                wq_sb4 = wtmp.tile([128, CH, C], BF16, tag="wq4")
                wp_sb = consts.tile([128, CH, C], BF16, tag="wp")
                nc.sync.dma_start(out=wk_sb4[:], in_=wk_d)
                nc.gpsimd.dma_start(out=wv_sb4[:], in_=wv_d)
                nc.sync.dma_start(out=wq_sb4[:], in_=wq_d)
                # gate the late-needed 2.6MB (X32 + wp) behind the last x
                # slice: this ACT op stalls until x7 lands, so the transfers
                # below cannot steal HBM bandwidth from the x stream
                nc.scalar.activation(out=tscr[:], in_=X_tiles[NS - 1][:, 0, 0:1],
                                     func=mybir.ActivationFunctionType.Copy)
                nc.scalar.dma_start(out=X32_sb[:], in_=x32_r[:])
                nc.scalar.dma_start(out=wp_sb[:], in_=wp_d)

                for ch in range(CH):
                    nc.vector.bn_aggr(out=mv[:, ch, :], in_=st_acc[:, ch, :, :])
                nc.vector.tensor_reduce(
                    out=sm2[:], in_=sma[:], axis=mybir.AxisListType.X,
                    op=mybir.AluOpType.add,
                )
                nc.vector.tensor_reduce(
                    out=sq2[:], in_=sqa[:], axis=mybir.AxisListType.X,
                    op=mybir.AluOpType.add,
                )
                # combine the two partial stats into per-channel sums (the
                # 1/(16*N) group-average normalization is folded into gmat on
                # the host, keeping this chain DVE-only -- no ACT hops)
                W1 = float(NSV * 512)
                nc.vector.scalar_tensor_tensor(
                    out=gs[:, :, 0], in0=mv[:, :, 0], scalar=W1, in1=sm2[:, :, 0],
                    op0=mybir.AluOpType.mult, op1=mybir.AluOpType.add,
                )
                nc.vector.tensor_mul(out=q1[:], in0=mv[:, :, 0], in1=mv[:, :, 0])
                nc.vector.tensor_add(out=q1[:], in0=q1[:], in1=mv[:, :, 1])
                nc.vector.scalar_tensor_tensor(
                    out=gs[:, :, 1], in0=q1[:], scalar=W1, in1=sq2[:, :, 0],
                    op0=mybir.AluOpType.mult, op1=mybir.AluOpType.add,
                )
                pg = psc.tile([128, CH, 2], F32, tag="pc", bufs=4)
                nc.tensor.matmul(pg[:], gm_sb[:], gs[:], start=True, stop=True)
                nc.vector.tensor_copy(out=mean_sb[:], in_=pg[:, :, 0])
                nc.vector.tensor_mul(out=rstd_sb[:], in0=mean_sb[:], in1=mean_sb[:])
                nc.vector.tensor_sub(out=rstd_sb[:], in0=pg[:, :, 1], in1=rstd_sb[:])
                nc.scalar.activation(
                    out=rstd_sb[:], in_=rstd_sb[:],
                    func=mybir.ActivationFunctionType.Sqrt, bias=eps_sb[:],
                )
                nc.vector.reciprocal(out=rstd_sb[:], in_=rstd_sb[:])

                # ---- fold groupnorm into weights: w8 = fp8(w * rstd(c_in));
                # the convs then also run as fp8 DoubleRow (single rounding:
                # bf16 host weights are scaled and quantized in one DVE op)
                wk8 = wtmp.tile([128, CH, C], FP8, tag="wk8")
                wv8 = wtmp.tile([128, CH, C], FP8, tag="wv8")
                wq8 = wtmp.tile([128, CH, C], FP8, tag="wq8")
                for w8, w_sb4x in ((wk8, wk_sb4), (wv8, wv_sb4), (wq8, wq_sb4)):
                    for ch in range(CH):
                        nc.vector.tensor_scalar_mul(
                            out=w8[:, ch, :], in0=w_sb4x[:, ch, :],
                            scalar1=rstd_sb[:, ch:ch + 1],
                        )

                mean_r = stats.tile([128, CH], FP8, tag="meanr")
                nc.vector.tensor_copy(out=mean_r[:], in_=mean_sb[:])
                # dq[o] = sum_c wq'[c,o] mu[c] directly in per-partition layout
                # (o on partitions) via 16 N=1 transposed matmuls; same for dv
                pdq = psc.tile([128, CH], F32, tag="pc", name="pdq" + sfx)
                pdv = psc.tile([128, CH], F32, tag="pc", name="pdv" + sfx)
                for pd, w8 in ((pdq, wq8), (pdv, wv8)):
                    for t in range(CH):
                        for ch in range(CH):
                            nc.tensor.matmul(
                                pd[:, t:t + 1],
                                w8[:, ch, t * 128:(t + 1) * 128],
                                mean_r[:, ch:ch + 1],
                                start=(ch == 0), stop=(ch == CH - 1),
                            )
                bcorr_q = small.tile([128, CH], F32, tag="bcorrq")
                nc.vector.tensor_sub(out=bcorr_q[:], in0=bq_sb[:], in1=pdq[:])
                dv_sb = small.tile([128, CH], BF16, tag="dv")
                nc.vector.tensor_copy(out=dv_sb[:], in_=pdv[:])

                # ---- phase 2: K / V^T / Q convs, fp8 DoubleRow ----
                for s in range(NS):
                    xsl = X8[s]
                    # K[c_out, j_slice]; no bias (cancels in softmax rows)
                    for t in range(CH):
                        pk = psc.tile([128, 512], F32, tag="pc")
                        for ch2 in range(0, CH, 2):
                            nc.tensor.matmul(
                                pk[:], wk8[:, ch2:ch2 + 2, t * 128:(t + 1) * 128],
                                xsl[:, ch2:ch2 + 2, :],
                                start=(ch2 == 0), stop=(ch2 == CH - 2),
                                perf_mode=mybir.MatmulPerfMode.DoubleRow,
                            )
                        nc.scalar.copy(
                            out=K_sb[:, t, s * 512:(s + 1) * 512], in_=pk[:],
                        )
                    # V^T[j_tile, c] resident in SBUF; bias applied via bp fold
                    for jj in range(4):
                        jt = 4 * s + jj
                        pv = psc.tile([128, 512], F32, tag="pc")
                        for ch2 in range(0, CH, 2):
                            nc.tensor.matmul(
                                pv[:], xsl[:, ch2:ch2 + 2, jj * 128:(jj + 1) * 128],
                                wv8[:, ch2:ch2 + 2, :],
                                start=(ch2 == 0), stop=(ch2 == CH - 2),
                                perf_mode=mybir.MatmulPerfMode.DoubleRow,
                            )
                        nc.vector.tensor_copy(out=VT_sb[:, jt, :], in_=pv[:])
                    # Q convs ride mid-sweep
                    if s == 3:
                        for sq in range(IC):
                            for t in range(CH):
                                pq = psc.tile([128, 512], F32, tag="pd", name=f"pq_{sq}_{t}" + sfx, bufs=2)
                                for ch2 in range(0, CH, 2):
                                    nc.tensor.matmul(
                                        pq[:], wq8[:, ch2:ch2 + 2, t * 128:(t + 1) * 128],
                                        X8[sq][:, ch2:ch2 + 2, :],
                                        start=(ch2 == 0), stop=(ch2 == CH - 2),
                                        perf_mode=mybir.MatmulPerfMode.DoubleRow,
                                    )
                                nc.vector.tensor_scalar_add(
                                    out=Q_sb[:, t, sq * 512:(sq + 1) * 512], in0=pq[:],
                                    scalar1=bcorr_q[:, t:t + 1],
                                )

            # effective proj bias: bp (with host-folded V bias) minus the
            # runtime wp^T (wv'^T mu) correction, in per-partition layout
            pcorr = psc.tile([128, CH], F32, tag="pc", name="pcorr" + sfx)
            for t in range(CH):
                for ch in range(CH):
                    nc.tensor.matmul(
                        pcorr[:, t:t + 1],
                        wp_sb[:, ch, t * 128:(t + 1) * 128],
                        dv_sb[:, ch:ch + 1],
                        start=(ch == 0), stop=(ch == CH - 1),
                    )
            bp_eff = small.tile([128, CH], F32, tag="bpeff")
            nc.vector.tensor_sub(out=bp_eff[:], in0=bp_sb[:], in1=pcorr[:])

            # ---- phase 3: attention + proj, per i-chunk of 512 ----
            with (
                tc.tile_pool(name="pexp" + sfx + sfx, bufs=4) as pexp,
                tc.tile_pool(name="osb" + sfx + sfx, bufs=4) as osb,
            ):
                ps_tiles = {}
                pt_done = {}
                emitted = set()
                NPAIR = JT // 2

                def emit_s(ic, pr):
                    # one S-pair: two j-tiles into a double-wide (2-bank) psum;
                    # fp8 DoubleRow contracts two 128-channel chunks per matmul
                    emitted.add((ic, pr))
                    ps = psc.tile([128, 2, 512], F32, tag="pd", name=f"ps_{ic}_{pr}" + sfx, bufs=2)
                    for u in range(2):
                        jt = 2 * pr + u
                        for ch2 in range(0, CH, 2):
                            nc.tensor.matmul(
                                ps[:, u, :],
                                K_sb[:, ch2:ch2 + 2, jt * 128:(jt + 1) * 128],
                                Q_sb[:, ch2:ch2 + 2, ic * 512:(ic + 1) * 512],
                                start=(ch2 == 0), stop=(ch2 == CH - 2),
                                perf_mode=mybir.MatmulPerfMode.DoubleRow,
                            )
                    ps_tiles[(ic, pr)] = ps

                emit_s(0, 0)
                for ic in range(IC):
                    po = [
                        psc.tile([128, 512], F32, tag="pc", name=f"po_{ic}_{ct}" + sfx)
                        for ct in range(CH)
                    ]
                    rs_parts = [
                        small.tile([128, 512], BF16, tag=f"rsacc{k}", name=f"rs_{ic}_{k}" + sfx)
                        for k in range(2)
                    ]

                    # software-pipelined: emit S(pr+1) before O(pr) so the PE
                    # never waits on the ACT exp of the current tile; at the
                    # end of a chunk, prefetch the next chunk's first S tiles
                    # so the PE has work during the DVE-heavy epilogue
                    for pr in range(NPAIR):
                        # one exp instruction covers both j-tiles of the pair
                        # (pair 0 may have been prefetched by the previous
                        # chunk's epilogue)
                        if (ic, pr) in pt_done:
                            pt = pt_done.pop((ic, pr))
                        else:
                            pt = pexp.tile([128, 2, 512], FP8, tag="pt", name=f"pt_{ic}_{pr}" + sfx)
                            nc.scalar.activation(
                                out=pt[:], in_=ps_tiles.pop((ic, pr))[:],
                                func=mybir.ActivationFunctionType.Exp, scale=SCALE,
                                bias=expb[:],
                            )
                        if pr + 1 < NPAIR:
                            if (ic, pr + 1) not in emitted:
                                emit_s(ic, pr + 1)
                        elif ic + 1 < IC:
                            emit_s(ic + 1, 0)
                        # O: fp8 DoubleRow contracts the pair's two j-tiles
                        # (256 positions) in one matmul per output chunk
                        for ct in range(CH):
                            nc.tensor.matmul(
                                po[ct][:],
                                VT_sb[:, 2 * pr:2 * pr + 2, ct * 128:(ct + 1) * 128],
                                pt[:],
                                start=(pr == 0), stop=(pr == NPAIR - 1),
                                perf_mode=mybir.MatmulPerfMode.DoubleRow,
                            )
                        for u in range(2):
                            rs_k = rs_parts[u]
                            if pr == 0:
                                nc.vector.tensor_copy(out=rs_k[:], in_=pt[:, u, :])
                            else:
                                nc.vector.tensor_add(out=rs_k[:], in0=rs_k[:], in1=pt[:, u, :])

                    # cross-partition row sums broadcast to all partitions with
                    # one ones[128,128] matmul, then one [128,512] reciprocal
                    nc.vector.tensor_add(
                        out=rs_parts[0][:], in0=rs_parts[0][:], in1=rs_parts[1][:]
                    )
                    pbs = psc.tile([128, 512], F32, tag="pc", name=f"pbs_{ic}" + sfx)
                    nc.tensor.matmul(pbs[:], ones128[:], rs_parts[0][:], start=True, stop=True)
                    rinv_bc = small.tile([128, 512], F32, tag="rinvbc")
                    nc.vector.reciprocal(out=rinv_bc[:], in_=pbs[:])

                    # O evictions all on the scalar engine so the (slow) DVE
                    # reciprocal cannot block the proj matmuls
                    o_sb = []
                    for ct in range(CH):
                        ot = osb.tile([128, 512], BF16, tag="ot", name=f"ot_{ic}_{ct}" + sfx)
                        nc.scalar.copy(out=ot[:], in_=po[ct][:])
                        o_sb.append(ot)
                    # prefetch the next chunk's first exp right behind the o
                    # evictions so its O matmuls never wait on a cold ACT queue
                    if ic + 1 < IC:
                        ptn = pexp.tile([128, 2, 512], FP8, tag="pt", name=f"pt_{ic + 1}_0" + sfx)
                        nc.scalar.activation(
                            out=ptn[:], in_=ps_tiles.pop((ic + 1, 0))[:],
                            func=mybir.ActivationFunctionType.Exp, scale=SCALE,
                            bias=expb[:],
                        )
                        pt_done[(ic + 1, 0)] = ptn

                    # proj + normalize + residual (fp32 x slice); the residual
                    # add runs on gpsimd so the DVE reciprocal can't stall it
                    for ct in range(CH):
                        py = psc.tile([128, 512], F32, tag="pc", name=f"py_{ic}_{ct}" + sfx)
                        for ch in range(CH):
                            nc.tensor.matmul(
                                py[:], wp_sb[:, ch, ct * 128:(ct + 1) * 128],
                                o_sb[ch][:], start=(ch == 0), stop=(ch == CH - 1),
                            )
                        ft = fin.tile([128, 512], F32, tag="ft", name=f"ft_{ic}_{ct}" + sfx)
                        nc.vector.tensor_mul(out=ft[:], in0=py[:], in1=rinv_bc[:])
                        nc.vector.scalar_tensor_tensor(
                            out=ft[:],
                            in0=X32_sb[:, ct, ic * 512:(ic + 1) * 512],
                            scalar=bp_eff[:, ct:ct + 1],
                            in1=ft[:],
                            op0=mybir.AluOpType.add,
                            op1=mybir.AluOpType.add,
                        )
                        nc.sync.dma_start(
                            out=out_r[:, ct, ic * 512:(ic + 1) * 512], in_=ft[:],
                        )

            ctx_psum.close()

    nc.compile()
    return nc


def _prepare_inputs(x, gn_scale, gn_bias, wq, bq, wk, bk, wv, bv, wp, bp):
    import ml_dtypes
    x = np.asarray(x, np.float32)
    gn_scale = np.asarray(gn_scale, np.float32)
    gn_bias = np.asarray(gn_bias, np.float32)

    def fold(w, b):
        w = np.asarray(w, np.float32)
        b = np.asarray(b, np.float32)
        return w * gn_scale[None, :], b + w @ gn_bias

    wq2, bq2 = fold(wq, bq)
    wk2, _ = fold(wk, bk)          # K bias cancels in softmax -> dropped
    wv2, bv2 = fold(wv, bv)
    wp2 = np.asarray(wp, np.float32)
    # V bias is a per-channel constant on the attention output: push it
    # through the projection into bp on the host
    bp2 = np.asarray(bp, np.float32) + wp2 @ bv2

    # gmat averages 16 channels per group AND carries the 1/N spatial
    # normalization (the device-side stats chain produces raw sums)
    gmat = np.zeros((128, 128), np.float32)
    for g in range(8):
        gmat[g * 16:(g + 1) * 16, g * 16:(g + 1) * 16] = 1.0 / (16.0 * N)

    bf = ml_dtypes.bfloat16

    def pmaj(a):
        # [C, k] -> [128, CH, k] partition-major (channel c = ch*128 + p)
        return np.ascontiguousarray(a.reshape(CH, 128, -1).transpose(1, 0, 2))

    shared = {
        "wqt": pmaj(wq2.T.astype(bf)),
        "wkt": pmaj(wk2.T.astype(bf)),
        "wvt": pmaj(wv2.T.astype(bf)),
        "wpt": pmaj(wp2.T.astype(bf)),
        "bq": np.ascontiguousarray(bq2.reshape(CH, 128).T),
        "bp": np.ascontiguousarray(bp2.reshape(CH, 128).T),
        "gmat": gmat,
    }

    xf = x.reshape(B, C, N)
    in_maps = []
    for core in range(8):
        b, qc = divmod(core, 4)
        i0 = qc * I
        xb = xf[b]
        xperm = np.concatenate([xb[:, i0:i0 + I], xb[:, :i0], xb[:, i0 + I:]], axis=1)
        # x: [128, NS, CH, 512] slice-major so each slice DMA has 4KB lines;
        # x8 is the fp8 conv operand, quantized from the same bf16 values
        xp = np.ascontiguousarray(
            pmaj(xperm.astype(bf)).reshape(128, CH, NS, 512).transpose(0, 2, 1, 3))
        in_maps.append({
            "x": xp,
            "x8": np.ascontiguousarray(xp.astype(ml_dtypes.float8_e4m3)),
            "x32": pmaj(xb[:, i0:i0 + I]),
            **shared,
        })
    return in_maps


def _run(in_maps, trace=False):
    if "nc" not in _cached:
        _cached["nc"] = _build()
    return run_bass_kernel_spmd(_cached["nc"], in_maps, list(range(8)), trace=trace)


def kernel(x, gn_scale, gn_bias, wq, bq, wk, bk, wv, bv, wp, bp):
    in_maps = _prepare_inputs(x, gn_scale, gn_bias, wq, bq, wk, bk, wv, bv, wp, bp)
    res = _run(in_maps)
    out = np.empty((B, C, N), np.float32)
    for core in range(8):
        b, qc = divmod(core, 4)
        out[b][:, qc * I:(qc + 1) * I] = res.results[core]["out"]
    return out.reshape(B, C, H, W)


# revision 53
# speedup vs baseline: 1.0069x; 1.0069x over previous
"""AttnBlock (GroupNorm -> single-head 4096x4096 attention -> proj -> residual)
on x:[2,512,64,64] f32, distributed over 8 trn2 NeuronCores.

Sharding: data-parallel over batch (2) x sequence-parallel over query rows
(4 chunks of 1024). Each core receives its batch's full [512, 4096] image with
spatial columns permuted so that its own 1024 query positions are columns
0:1024 (attention and groupnorm are permutation-invariant over spatial
positions, which keeps the SPMD program identical across cores).

Precision ladder (fp32 accumulation in PSUM throughout): x streams in as
bf16 (for the GroupNorm stats) plus a host-prequantized fp8 e4m3 copy for
the convs; the q/k/v conv and attention matmuls (S=K^T Q, O=V^T P) all run
as fp8 DoubleRow matmuls -- 256-deep contraction per instruction at the
fp8 2x rate -- and V^T stays SBUF-resident (no DRAM spill/reload). Only
the proj matmul and the stats stay bf16. exp is computed as
exp(s*scale - 5): softmax is invariant to the row-constant shift and it
keeps exp outputs inside e4m3 range (e4m3 max is 448; unshifted exp
overflows to NaN). A numpy simulation of this quantization through the
reference gives 7.5e-3 absmax relative error (gate is 2e-2; 6.7e-3
measured on hardware). The residual path reads a separate fp32 copy of
the core's own 1024 columns.

The x DMA is striped over three queues (sync/gpsimd/scalar) with all issue
instructions emitted before any compute op, weights queue behind the x
slices, and the late-needed residual/proj-weight transfers are gated behind
the last x slice via a tiny ACT dependency op. GroupNorm stats are split
DVE (bn_stats, 6 slices) / ACT (Square+Copy accum_out, first 2 slices to
arrive), combined as raw sums with the group-average and 1/N normalization
folded into the host-built gmat.

GroupNorm is folded into the q/k/v weights on device (h = (x-mu)*rstd, so
W^T rows are scaled by rstd). Bias handling exploits softmax structure:
  - K bias (and its -W^T mu correction) adds a per-row constant to the
    logits -> cancels in softmax -> skipped entirely.
  - V bias is a per-channel constant on the attention output -> folded into
    the proj bias ON HOST (bp += wp @ bv); only the runtime -W_v^T mu part
    is applied on device, pushed through wp into the final bias with 16
    tiny transposed matmuls.
  - Q bias is applied at Q eviction; its correction (bq - W_q^T mu) is
    computed with 16 tiny transposed matmuls directly in per-partition
    layout (no DRAM-roundtrip transpose).
Softmax normalization: exp tiles accumulate on DVE into bf16 row-partials;
one ones[128,128] matmul broadcasts the cross-partition row sums to all
partitions, and a [128,512] DVE reciprocal feeds the final scale -- all off
the PE critical path.
"""

import numpy as np

import concourse.bass as bass
import concourse.mybir as mybir
import concourse.tile as tile
from concourse import bacc
from concourse.bass_utils import run_bass_kernel_spmd

F32 = mybir.dt.float32
BF16 = mybir.dt.bfloat16
FP8 = mybir.dt.float8e4

B = 2
C = 512
H = 64
W = 64
N = H * W            # 4096 spatial positions
G = 32               # groups
EPS = 1e-6
CH = 4               # channel chunks of 128
NS = 8               # j slices of 512
JT = 32              # j tiles of 128
I = 1024             # query positions per core
IC = 2               # i chunks of 512 per core
SCALE = float(C) ** -0.5

_cached = {}


def _build(repeat=1):
    nc = bacc.Bacc("TRN2", target_bir_lowering=False, debug=False, num_devices=8)

    # all inputs are host-relayouted to partition-major so every DMA is a
    # straight [128, k] copy with fully contiguous per-partition lines
    x_d = nc.dram_tensor("x", [128, NS, CH, 512], BF16, kind="ExternalInput").ap()
    x8_d = nc.dram_tensor("x8", [128, NS, CH, 512], FP8, kind="ExternalInput").ap()
    x32_d = nc.dram_tensor("x32", [128, CH, I], F32, kind="ExternalInput").ap()
    wq_d = nc.dram_tensor("wqt", [128, CH, C], BF16, kind="ExternalInput").ap()
    wk_d = nc.dram_tensor("wkt", [128, CH, C], BF16, kind="ExternalInput").ap()
    wv_d = nc.dram_tensor("wvt", [128, CH, C], BF16, kind="ExternalInput").ap()
    wp_d = nc.dram_tensor("wpt", [128, CH, C], BF16, kind="ExternalInput").ap()
    bq_d = nc.dram_tensor("bq", [128, CH], F32, kind="ExternalInput").ap()
    bp_d = nc.dram_tensor("bp", [128, CH], F32, kind="ExternalInput").ap()
    gm_d = nc.dram_tensor("gmat", [128, 128], F32, kind="ExternalInput").ap()
    out_ds = [
        nc.dram_tensor("out" if r == 0 else f"out{r}", [C, I], F32,
                       kind="ExternalOutput").ap()
        for r in range(repeat)
    ]

    x_r = x_d       # [128, 8, 4, 512] slice-major: 4KB lines per slice DMA
    x32_r = x32_d   # [128, 4, 1024]

    with tile.TileContext(nc) as tc:
      for rep in range(repeat):
        sfx = f"_{rep}"
        out_r = out_ds[rep].rearrange("(ch p) i -> p ch i", p=128)
        from contextlib import ExitStack
        ctx_psum = ExitStack()
        with (
            tc.tile_pool(name="consts" + sfx + sfx, bufs=1) as consts,
            tc.tile_pool(name="big" + sfx + sfx, bufs=1) as big,
            tc.tile_pool(name="stats" + sfx + sfx, bufs=1) as stats,
            tc.tile_pool(name="small" + sfx + sfx, bufs=1) as small,
            tc.tile_pool(name="fin" + sfx + sfx, bufs=2) as fin,
        ):
            # ---- persistent constants (on the vector DMA queue, off the
            # x-critical sync/gpsimd queues) ----
            bp_sb = consts.tile([128, CH], F32, tag="bp")
            nc.scalar.dma_start(out=bp_sb[:], in_=bp_d)
            bq_sb = consts.tile([128, CH], F32, tag="bq")
            nc.scalar.dma_start(out=bq_sb[:], in_=bq_d)
            gm_sb = consts.tile([128, 128], F32, tag="gm")
            nc.scalar.dma_start(out=gm_sb[:], in_=gm_d)
            eps_sb = consts.tile([128, 1], F32, tag="eps")
            nc.vector.memset(eps_sb[:], EPS)
            # global logit shift: exp(s*scale - 5) keeps exp outputs inside
            # fp8 e4m3 range (max 448; logits reach ~5 sigma). Softmax is
            # invariant to a row-constant shift and the uniform e^-5 factor
            # cancels exactly in the rowsum normalization.
            expb = consts.tile([128, 1], F32, tag="expb")
            nc.vector.memset(expb[:], -5.0)
            ones128 = consts.tile([128, 128], BF16, tag="ones128")
            nc.vector.memset(ones128[:], 1.0)
            tscr = consts.tile([128, 1], F32, tag="tscr")

            X_tiles = [
                big.tile([128, CH, 512], BF16, tag=f"X{s}", name=f"X{s}" + sfx)
                for s in range(NS)
            ]  # bf16 x, resident, one tile per j-slice for fine-grained deps
            X32_sb = big.tile([128, CH, I], F32, tag="X32")  # fp32 residual slice
            # attention storage is fp8 e4m3: S and O run as DoubleRow matmuls
            # (256-deep contraction per instruction, half the PE cycles);
            # numpy sim of this quantization: 3.8e-3 absmax rel (gate 2e-2)
            K_sb = big.tile([128, CH, N], FP8, tag="K")      # K[c, j]
            Q_sb = big.tile([128, CH, I], FP8, tag="Q")      # Q[c, i]
            VT_sb = big.tile([128, JT, C], FP8, tag="VT")    # V^T[j, c] resident

            # groupnorm stats split across three engines, ownership matched
            # to DMA arrival order (slices 0/1/2 land first, one per queue):
            # ACT 2-pass accum on (0,3), GpSimd STT accum on (2,), DVE
            # bn_stats on the rest
            OWN_ACT = {0: 0, 1: 1}        # slice -> accumulator slot
            OWN_GP = {}
            NSV = NS - len(OWN_ACT) - len(OWN_GP)
            NSA = len(OWN_ACT) + len(OWN_GP)
            st_acc = stats.tile([128, CH, NSV, 6], F32, tag="stacc")
            sma = stats.tile([128, CH, NSA], F32, tag="sma")
            sqa = stats.tile([128, CH, NSA], F32, tag="sqa")
            sm2 = stats.tile([128, CH, 1], F32, tag="sm2")
            sq2 = stats.tile([128, CH, 1], F32, tag="sq2")
            scr = stats.tile([128, 512], BF16, tag="scr")
            scrg = stats.tile([128, 512], BF16, tag="scrg")
            mv = stats.tile([128, CH, 2], F32, tag="mv")
            q1 = stats.tile([128, CH], F32, tag="q1")
            gs = stats.tile([128, CH, 2], F32, tag="gs")
            mean_sb = stats.tile([128, CH], F32, tag="mean")
            rstd_sb = stats.tile([128, CH], F32, tag="rstd")

            psc = ctx_psum.enter_context(
                tc.tile_pool(name="psc" + sfx + sfx, bufs=4, space="PSUM")
            )
            with tc.tile_pool(name="wtmp" + sfx + sfx, bufs=1) as wtmp:
                # ---- phase 1: groupnorm stats over streaming bf16 x ----
                # x is striped over three DMA queues so it gets the full HBM
                # bandwidth; everything else queues behind the x slices
                nsv_i = 0
                qs = (nc.sync, nc.gpsimd, nc.scalar)
                # issue every x-slice DMA first: each engine's issue runs
                # before any of its (slow) stats ops, so the three queues
                # stream x back-to-back at full bandwidth. The fp8 conv copy
                # of x (host-quantized from the same bf16 values) queues
                # right behind the bf16 stream.
                X8 = [
                    big.tile([128, CH, 512], FP8, tag=f"X8{s}", name=f"X8{s}" + sfx)
                    for s in range(NS)
                ]
                for s in range(NS):
                    qs[s % 3].dma_start(out=X_tiles[s][:], in_=x_r[:, s, :, :])
                for s in range(NS):
                    qs[s % 3].dma_start(out=X8[s][:], in_=x8_d[:, s, :, :])
                for s in range(NS):
                    if s in OWN_ACT:
                        a = OWN_ACT[s]
                        for ch in range(CH):
                            nc.scalar.activation(
                                out=scr[:], in_=X_tiles[s][:, ch, :],
                                func=mybir.ActivationFunctionType.Square,
                                accum_out=sqa[:, ch, a:a + 1],
                            )
                            nc.scalar.activation(
                                out=scr[:], in_=X_tiles[s][:, ch, :],
                                func=mybir.ActivationFunctionType.Copy,
                                accum_out=sma[:, ch, a:a + 1],
                            )
                    elif s in OWN_GP:
                        a = OWN_GP[s]
                        for ch in range(CH):
                            nc.gpsimd.scalar_tensor_tensor(
                                out=scrg[:], in0=X_tiles[s][:, ch, :], scalar=1.0,
                                in1=X_tiles[s][:, ch, :],
                                op0=mybir.AluOpType.mult, op1=mybir.AluOpType.mult,
                                accum_out=sqa[:, ch, a:a + 1],
                            )
                            nc.gpsimd.scalar_tensor_tensor(
                                out=scrg[:], in0=X_tiles[s][:, ch, :], scalar=1.0,
                                in1=ones512[:],
                                op0=mybir.AluOpType.mult, op1=mybir.AluOpType.mult,
                                accum_out=sma[:, ch, a:a + 1],
                            )
                    else:
                        for ch in range(CH):
                            nc.vector.bn_stats(
                                out=st_acc[:, ch, nsv_i, :], in_=X_tiles[s][:, ch, :],
                            )
                        nsv_i += 1
                # prewarm the Sqrt ACT table now (runs right after the last
                # ACT stats op, overlapping the DVE aggregation chain)
                nc.scalar.activation(out=tscr[:], in_=eps_sb[:],
                                     func=mybir.ActivationFunctionType.Sqrt)
                # weights queue behind the x slices on sync; the residual fp32
                # slice and proj weight queue behind the x slices on gpsimd.
                # Each queue drains in order, so x gets the full HBM bandwidth
                # first and the late-needed tensors stream under the convs.
                wk_sb4 = wtmp.tile([128, CH, C], BF16, tag="wk4")
                wv_sb4 = wtmp.tile([128, CH, C], BF16, tag="wv4")
                wq_sb4 = wtmp.tile([128, CH, C], BF16, tag="wq4")
                wp_sb = consts.tile([128, CH, C], BF16, tag="wp")
                nc.sync.dma_start(out=wk_sb4[:], in_=wk_d)
                nc.gpsimd.dma_start(out=wv_sb4[:], in_=wv_d)
                nc.sync.dma_start(out=wq_sb4[:], in_=wq_d)
                # gate the late-needed 2.6MB (X32 + wp) behind the last x
                # slice: this ACT op stalls until x7 lands, so the transfers
                # below cannot steal HBM bandwidth from the x stream
                nc.scalar.activation(out=tscr[:], in_=X_tiles[NS - 1][:, 0, 0:1],
                                     func=mybir.ActivationFunctionType.Copy)
                nc.scalar.dma_start(out=X32_sb[:], in_=x32_r[:])
                nc.scalar.dma_start(out=wp_sb[:], in_=wp_d)

                for ch in range(CH):
                    nc.vector.bn_aggr(out=mv[:, ch, :], in_=st_acc[:, ch, :, :])
                nc.vector.tensor_reduce(
                    out=sm2[:], in_=sma[:], axis=mybir.AxisListType.X,
                    op=mybir.AluOpType.add,
                )
                nc.vector.tensor_reduce(
                    out=sq2[:], in_=sqa[:], axis=mybir.AxisListType.X,
                    op=mybir.AluOpType.add,
                )
                # combine the two partial stats into per-channel sums (the
                # 1/(16*N) group-average normalization is folded into gmat on
                # the host, keeping this chain DVE-only -- no ACT hops)
                W1 = float(NSV * 512)
                nc.vector.scalar_tensor_tensor(
                    out=gs[:, :, 0], in0=mv[:, :, 0], scalar=W1, in1=sm2[:, :, 0],
                    op0=mybir.AluOpType.mult, op1=mybir.AluOpType.add,
                )
                nc.vector.tensor_mul(out=q1[:], in0=mv[:, :, 0], in1=mv[:, :, 0])
                nc.vector.tensor_add(out=q1[:], in0=q1[:], in1=mv[:, :, 1])
                nc.vector.scalar_tensor_tensor(
                    out=gs[:, :, 1], in0=q1[:], scalar=W1, in1=sq2[:, :, 0],
                    op0=mybir.AluOpType.mult, op1=mybir.AluOpType.add,
                )
                pg = psc.tile([128, CH, 2], F32, tag="pc", bufs=4)
                nc.tensor.matmul(pg[:], gm_sb[:], gs[:], start=True, stop=True)
                nc.vector.tensor_copy(out=mean_sb[:], in_=pg[:, :, 0])
                nc.vector.tensor_mul(out=rstd_sb[:], in0=mean_sb[:], in1=mean_sb[:])
                nc.vector.tensor_sub(out=rstd_sb[:], in0=pg[:, :, 1], in1=rstd_sb[:])
                nc.scalar.activation(
                    out=rstd_sb[:], in_=rstd_sb[:],
                    func=mybir.ActivationFunctionType.Sqrt, bias=eps_sb[:],
                )
                nc.vector.reciprocal(out=rstd_sb[:], in_=rstd_sb[:])

                # ---- fold groupnorm into weights: w8 = fp8(w * rstd(c_in));
                # the convs then also run as fp8 DoubleRow (single rounding:
                # bf16 host weights are scaled and quantized in one DVE op)
                wk8 = wtmp.tile([128, CH, C], FP8, tag="wk8")
                wv8 = wtmp.tile([128, CH, C], FP8, tag="wv8")
                wq8 = wtmp.tile([128, CH, C], FP8, tag="wq8")
                for w8, w_sb4x in ((wk8, wk_sb4), (wv8, wv_sb4), (wq8, wq_sb4)):
                    for ch in range(CH):
                        nc.vector.tensor_scalar_mul(
                            out=w8[:, ch, :], in0=w_sb4x[:, ch, :],
                            scalar1=rstd_sb[:, ch:ch + 1],
                        )

                mean_r = stats.tile([128, CH], FP8, tag="meanr")
                nc.vector.tensor_copy(out=mean_r[:], in_=mean_sb[:])
                # dq[o] = sum_c wq'[c,o] mu[c] directly in per-partition layout
                # (o on partitions) via 16 N=1 transposed matmuls; same for dv
                pdq = psc.tile([128, CH], F32, tag="pc", name="pdq" + sfx)
                pdv = psc.tile([128, CH], F32, tag="pc", name="pdv" + sfx)
                for pd, w8 in ((pdq, wq8), (pdv, wv8)):
                    for t in range(CH):
                        for ch in range(CH):
                            nc.tensor.matmul(
                                pd[:, t:t + 1],
                                w8[:, ch, t * 128:(t + 1) * 128],
                                mean_r[:, ch:ch + 1],
                                start=(ch == 0), stop=(ch == CH - 1),
                            )
                bcorr_q = small.tile([128, CH], F32, tag="bcorrq")
                nc.vector.tensor_sub(out=bcorr_q[:], in0=bq_sb[:], in1=pdq[:])
                dv_sb = small.tile([128, CH], BF16, tag="dv")
                nc.vector.tensor_copy(out=dv_sb[:], in_=pdv[:])

                # ---- phase 2: K / V^T / Q convs, fp8 DoubleRow ----
                for s in range(NS):
                    xsl = X8[s]
                    # K[c_out, j_slice]; no bias (cancels in softmax rows)
                    for t in range(CH):
                        pk = psc.tile([128, 512], F32, tag="pc")
                        for ch2 in range(0, CH, 2):
                            nc.tensor.matmul(
                                pk[:], wk8[:, ch2:ch2 + 2, t * 128:(t + 1) * 128],
                                xsl[:, ch2:ch2 + 2, :],
                                start=(ch2 == 0), stop=(ch2 == CH - 2),
                                perf_mode=mybir.MatmulPerfMode.DoubleRow,
                            )
                        nc.scalar.copy(
                            out=K_sb[:, t, s * 512:(s + 1) * 512], in_=pk[:],
                        )
                    # V^T[j_tile, c] resident in SBUF; bias applied via bp fold
                    for jj in range(4):
                        jt = 4 * s + jj
                        pv = psc.tile([128, 512], F32, tag="pc")
                        for ch2 in range(0, CH, 2):
                            nc.tensor.matmul(
                                pv[:], xsl[:, ch2:ch2 + 2, jj * 128:(jj + 1) * 128],
                                wv8[:, ch2:ch2 + 2, :],
                                start=(ch2 == 0), stop=(ch2 == CH - 2),
                                perf_mode=mybir.MatmulPerfMode.DoubleRow,
                            )
                        nc.vector.tensor_copy(out=VT_sb[:, jt, :], in_=pv[:])
                    # Q convs ride mid-sweep
                    if s == 3:
                        for sq in range(IC):
                            for t in range(CH):
                                pq = psc.tile([128, 512], F32, tag="pd", name=f"pq_{sq}_{t}" + sfx, bufs=2)
                                for ch2 in range(0, CH, 2):
                                    nc.tensor.matmul(
                                        pq[:], wq8[:, ch2:ch2 + 2, t * 128:(t + 1) * 128],
                                        X8[sq][:, ch2:ch2 + 2, :],
                                        start=(ch2 == 0), stop=(ch2 == CH - 2),
                                        perf_mode=mybir.MatmulPerfMode.DoubleRow,
                                    )
                                nc.vector.tensor_scalar_add(
                                    out=Q_sb[:, t, sq * 512:(sq + 1) * 512], in0=pq[:],
                                    scalar1=bcorr_q[:, t:t + 1],
                                )

            # effective proj bias: bp (with host-folded V bias) minus the
            # runtime wp^T (wv'^T mu) correction, in per-partition layout
            pcorr = psc.tile([128, CH], F32, tag="pc", name="pcorr" + sfx)
            for t in range(CH):
                for ch in range(CH):
                    nc.tensor.matmul(
                        pcorr[:, t:t + 1],
                        wp_sb[:, ch, t * 128:(t + 1) * 128],
                        dv_sb[:, ch:ch + 1],
                        start=(ch == 0), stop=(ch == CH - 1),
                    )
            bp_eff = small.tile([128, CH], F32, tag="bpeff")
            nc.vector.tensor_sub(out=bp_eff[:], in0=bp_sb[:], in1=pcorr[:])

            # ---- phase 3: attention + proj, per i-chunk of 512 ----
            with (
                tc.tile_pool(name="pexp" + sfx + sfx, bufs=4) as pexp,
                tc.tile_pool(name="osb" + sfx + sfx, bufs=4) as osb,
            ):
                ps_tiles = {}
                pt_done = {}
                emitted = set()
                NPAIR = JT // 2

                def emit_s(ic, pr):
                    # one S-pair: two j-tiles into a double-wide (2-bank) psum;
                    # fp8 DoubleRow contracts two 128-channel chunks per matmul
                    emitted.add((ic, pr))
                    ps = psc.tile([128, 2, 512], F32, tag="pd", name=f"ps_{ic}_{pr}" + sfx, bufs=2)
                    for u in range(2):
                        jt = 2 * pr + u
                        for ch2 in range(0, CH, 2):
                            nc.tensor.matmul(
                                ps[:, u, :],
                                K_sb[:, ch2:ch2 + 2, jt * 128:(jt + 1) * 128],
                                Q_sb[:, ch2:ch2 + 2, ic * 512:(ic + 1) * 512],
                                start=(ch2 == 0), stop=(ch2 == CH - 2),
                                perf_mode=mybir.MatmulPerfMode.DoubleRow,
                            )
                    ps_tiles[(ic, pr)] = ps

                emit_s(0, 0)
                for ic in range(IC):
                    po = [
                        psc.tile([128, 512], F32, tag="pc", name=f"po_{ic}_{ct}" + sfx)
                        for ct in range(CH)
                    ]
                    rs_parts = [
                        small.tile([128, 512], BF16, tag=f"rsacc{k}", name=f"rs_{ic}_{k}" + sfx)
                        for k in range(2)
                    ]

                    # software-pipelined: emit S(pr+1) before O(pr) so the PE
                    # never waits on the ACT exp of the current tile; at the
                    # end of a chunk, prefetch the next chunk's first S tiles
                    # so the PE has work during the DVE-heavy epilogue
                    for pr in range(NPAIR):
                        # one exp instruction covers both j-tiles of the pair
                        # (pair 0 may have been prefetched by the previous
                        # chunk's epilogue)
                        if (ic, pr) in pt_done:
                            pt = pt_done.pop((ic, pr))
                        else:
                            pt = pexp.tile([128, 2, 512], FP8, tag="pt", name=f"pt_{ic}_{pr}" + sfx)
                            nc.scalar.activation(
                                out=pt[:], in_=ps_tiles.pop((ic, pr))[:],
                                func=mybir.ActivationFunctionType.Exp, scale=SCALE,
                                bias=expb[:],
                            )
                        if pr + 1 < NPAIR:
                            if (ic, pr + 1) not in emitted:
                                emit_s(ic, pr + 1)
                        elif ic + 1 < IC:
                            emit_s(ic + 1, 0)
                        # O: fp8 DoubleRow contracts the pair's two j-tiles
                        # (256 positions) in one matmul per output chunk
                        for ct in range(CH):
                            nc.tensor.matmul(
                                po[ct][:],
                                VT_sb[:, 2 * pr:2 * pr + 2, ct * 128:(ct + 1) * 128],
                                pt[:],
                                start=(pr == 0), stop=(pr == NPAIR - 1),
                                perf_mode=mybir.MatmulPerfMode.DoubleRow,
                            )
                        for u in range(2):
                            rs_k = rs_parts[u]
                            if pr == 0:
                                nc.vector.tensor_copy(out=rs_k[:], in_=pt[:, u, :])
                            else:
                                nc.vector.tensor_add(out=rs_k[:], in0=rs_k[:], in1=pt[:, u, :])

                    # cross-partition row sums broadcast to all partitions with
                    # one ones[128,128] matmul, then one [128,512] reciprocal
                    nc.vector.tensor_add(
                        out=rs_parts[0][:], in0=rs_parts[0][:], in1=rs_parts[1][:]
                    )
                    pbs = psc.tile([128, 512], F32, tag="pc", name=f"pbs_{ic}" + sfx)
                    nc.tensor.matmul(pbs[:], ones128[:], rs_parts[0][:], start=True, stop=True)
                    rinv_bc = small.tile([128, 512], F32, tag="rinvbc")
                    nc.vector.reciprocal(out=rinv_bc[:], in_=pbs[:])

                    # O evictions all on the scalar engine so the (slow) DVE
                    # reciprocal cannot block the proj matmuls
                    o_sb = []
                    for ct in range(CH):
                        ot = osb.tile([128, 512], BF16, tag="ot", name=f"ot_{ic}_{ct}" + sfx)
                        nc.scalar.copy(out=ot[:], in_=po[ct][:])
                        o_sb.append(ot)
                    # prefetch the next chunk's first exp right behind the o
                    # evictions so its O matmuls never wait on a cold ACT queue
                    if ic + 1 < IC:
                        ptn = pexp.tile([128, 2, 512], FP8, tag="pt", name=f"pt_{ic + 1}_0" + sfx)
                        nc.scalar.activation(
                            out=ptn[:], in_=ps_tiles.pop((ic + 1, 0))[:],
                            func=mybir.ActivationFunctionType.Exp, scale=SCALE,
                            bias=expb[:],
                        )
                        pt_done[(ic + 1, 0)] = ptn

                    # proj + normalize + residual (fp32 x slice); the residual
                    # add runs on gpsimd so the DVE reciprocal can't stall it
                    for ct in range(CH):
                        py = psc.tile([128, 512], F32, tag="pc", name=f"py_{ic}_{ct}" + sfx)
                        for ch in range(CH):
                            nc.tensor.matmul(
                                py[:], wp_sb[:, ch, ct * 128:(ct + 1) * 128],
                                o_sb[ch][:], start=(ch == 0), stop=(ch == CH - 1),
                            )
                        ft = fin.tile([128, 512], F32, tag="ft", name=f"ft_{ic}_{ct}" + sfx)
                        nc.vector.tensor_mul(out=ft[:], in0=py[:], in1=rinv_bc[:])
                        nc.vector.scalar_tensor_tensor(
                            out=ft[:],
                            in0=X32_sb[:, ct, ic * 512:(ic + 1) * 512],
                            scalar=bp_eff[:, ct:ct + 1],
                            in1=ft[:],
                            op0=mybir.AluOpType.add,
                            op1=mybir.AluOpType.add,
                        )
                        nc.sync.dma_start(
                            out=out_r[:, ct, ic * 512:(ic + 1) * 512], in_=ft[:],
                        )

            ctx_psum.close()

    nc.compile()
    return nc


def _prepare_inputs(x, gn_scale, gn_bias, wq, bq, wk, bk, wv, bv, wp, bp):
    import ml_dtypes
    x = np.asarray(x, np.float32)
    gn_scale = np.asarray(gn_scale, np.float32)
    gn_bias = np.asarray(gn_bias, np.float32)

    def fold(w, b):
        w = np.asarray(w, np.float32)
        b = np.asarray(b, np.float32)
        return w * gn_scale[None, :], b + w @ gn_bias

    wq2, bq2 = fold(wq, bq)
    wk2, _ = fold(wk, bk)          # K bias cancels in softmax -> dropped
    wv2, bv2 = fold(wv, bv)
    wp2 = np.asarray(wp, np.float32)
    # V bias is a per-channel constant on the attention output: push it
    # through the projection into bp on the host
    bp2 = np.asarray(bp, np.float32) + wp2 @ bv2

    # gmat averages 16 channels per group AND carries the 1/N spatial
    # normalization (the device-side stats chain produces raw sums)
    gmat = np.zeros((128, 128), np.float32)
    for g in range(8):
        gmat[g * 16:(g + 1) * 16, g * 16:(g + 1) * 16] = 1.0 / (16.0 * N)

    bf = ml_dtypes.bfloat16

    def pmaj(a):
        # [C, k] -> [128, CH, k] partition-major (channel c = ch*128 + p)
        return np.ascontiguousarray(a.reshape(CH, 128, -1).transpose(1, 0, 2))

    shared = {
        "wqt": pmaj(wq2.T.astype(bf)),
        "wkt": pmaj(wk2.T.astype(bf)),
        "wvt": pmaj(wv2.T.astype(bf)),
        "wpt": pmaj(wp2.T.astype(bf)),
        "bq": np.ascontiguousarray(bq2.reshape(CH, 128).T),
        "bp": np.ascontiguousarray(bp2.reshape(CH, 128).T),
        "gmat": gmat,
    }

    xf = x.reshape(B, C, N)
    in_maps = []
    for core in range(8):
        b, qc = divmod(core, 4)
        i0 = qc * I
        xb = xf[b]
        xperm = np.concatenate([xb[:, i0:i0 + I], xb[:, :i0], xb[:, i0 + I:]], axis=1)
        # x: [128, NS, CH, 512] slice-major so each slice DMA has 4KB lines;
        # x8 is the fp8 conv operand, quantized from the same bf16 values
        xp = np.ascontiguousarray(
            pmaj(xperm.astype(bf)).reshape(128, CH, NS, 512).transpose(0, 2, 1, 3))
        in_maps.append({
            "x": xp,
            "x8": np.ascontiguousarray(xp.astype(ml_dtypes.float8_e4m3)),
            "x32": pmaj(xb[:, i0:i0 + I]),
            **shared,
        })
    return in_maps


def _run(in_maps, trace=False):
    if "nc" not in _cached:
        _cached["nc"] = _build()
    return run_bass_kernel_spmd(_cached["nc"], in_maps, list(range(8)), trace=trace)


def kernel(x, gn_scale, gn_bias, wq, bq, wk, bk, wv, bv, wp, bp):
    in_maps = _prepare_inputs(x, gn_scale, gn_bias, wq, bq, wk, bk, wv, bv, wp, bp)
    res = _run(in_maps)
    out = np.empty((B, C, N), np.float32)
    for core in range(8):
        b, qc = divmod(core, 4)
        out[b][:, qc * I:(qc + 1) * I] = res.results[core]["out"]
    return out.reshape(B, C, H, W)
